# revision 14
# baseline (speedup 1.0000x reference)
"""Trainium2 Bass kernel for multi-head attention with adaptive span masking.

Computation (per the nn.Module):
    q = (query @ Wq.T) split into B*H rows of size d=64
    attn = softmax((key . q + q @ key_pe) / sqrt(d))
    attn = renormalize(attn * adaptive_span_mask)
    out = (attn . value) merged heads @ Wo.T

Sharding: batch-parallel across 8 cores. Core c gets batches [4c, 4c+4)
(all 8 heads) = rows [32c, 32c+32) of key/value; Wq/Wo/key_pe/span are
replicated. Each core produces its own [4, 512] output block; the host
concatenates. No collectives needed.

Sparsity: the adaptive-span mask is exactly zero for m <= 8159 - span*M,
so only the suffix [m0_h, M) of each head's key/value rows is ever used.
m0_h is computed on the host from the span input (any span values give a
correct kernel; new values just trigger a rebuild) and the kernel only
loads/processes that suffix. The mask enters as a precomputed additive
8*ln(mask) bias inside the exp (exact for mask>0; -inf -> weight 0), and
the 1e-8*sum(exp) regularizer of the reference is dropped (~1e-6 relative).

Positional scores are computed as one dense PE matmul q @ key_pe in
[row, m] layout, then re-gridded to each head's [128, mo_h] block layout
with per-head SBUF->SBUF gather DMAs (cheap; avoids hundreds of tiny
PE weight loads).
"""

import math
import os
import sys

import numpy as np

for _p in ("/opt/trn_rl_repo", "/root/.axon_site/_ro/trn_rl_repo"):
    if os.path.isdir(_p) and _p not in sys.path:
        sys.path.insert(0, _p)

import concourse.bass as bass
import concourse.bacc as bacc
import concourse.mybir as mybir
from concourse.bass import ts
from concourse.masks import make_identity
from concourse.tile import TileContext

F32 = mybir.dt.float32

# Problem constants (hardcoded per contest contract)
NHEADS = 8
HEAD_DIM = 64
HID = NHEADS * HEAD_DIM  # 512
B = 32
M = 8192
RAMP = 32.0

N_CORES = 8
BPC = B // N_CORES        # 4 batches per core
NPC = BPC * NHEADS        # 32 (b,h) rows per core

_CACHE = {}


def _suffix_starts(span):
    """Per-head start offset m0 such that mask[h, m] == 0 for all m < m0.

    mask = clip((m - (M-1) + span*M)/RAMP + 1, 0, 1) is zero iff
    m <= (M-1) - RAMP - span*M. Align down to 128 with a small safety
    margin for float rounding.
    """
    s = np.asarray(span, np.float64).ravel()
    m0 = np.floor((M - 1) - RAMP - s * M).astype(np.int64) - 2
    m0 = np.clip(m0, 0, M - 128)
    m0 = (m0 // 128) * 128
    return tuple(int(x) for x in m0)


def build_nc(m0s):
    nc = bacc.Bacc(None, target_bir_lowering=False)
    AF = mybir.ActivationFunctionType
    ALU = mybir.AluOpType
    BF16 = mybir.dt.bfloat16

    mo_h = [(M - m0) // 128 for m0 in m0s]   # per-head blocks of 128 rows
    offs = [0] * NHEADS                      # per-head column offset in pos/masks
    for h in range(1, NHEADS):
        offs[h] = offs[h - 1] + mo_h[h - 1]
    SUM_MO = offs[-1] + mo_h[-1]
    m0_min = min(m0s)
    LALL = M - m0_min

    # Host permutes key/value rows to h-major (row r = h*BPC + b), so GB
    # consecutive rows share a head and can be fetched with one batched DMA.
    # Group order: biggest groups first (prefetch covers setup), then
    # alternate big/small so small-group fixed compute hides under big-group
    # DMAs and the tail is not a run of tiny groups.
    GB = 2
    # co-pairs (heads 2j,2j+1 feed output column chunk j) ordered by total
    # size descending; each pair finishes before the next pair's tail so its
    # output transpose can run mid-loop instead of serializing the tail.
    co_order = sorted(range(4), key=lambda c: -max(mo_h[2 * c], mo_h[2 * c + 1]))
    # schedule is a list of ("grp", h, g) and ("emit_co", c) steps. The
    # biggest pair's big head leads (prefetch depth); its small partner runs
    # dead last so the tail pipeline is as short as possible; every other
    # pair completes mid-loop so its output transpose overlaps the stream.
    c0 = co_order[0]
    ha0, hb0 = 2 * c0, 2 * c0 + 1
    if mo_h[ha0] < mo_h[hb0]:
        ha0, hb0 = hb0, ha0
    schedule = [("grp", ha0, 0), ("grp", ha0, 1)]
    for c in co_order[1:]:
        ha, hb = 2 * c, 2 * c + 1
        if mo_h[ha] < mo_h[hb]:
            ha, hb = hb, ha
        schedule += [("grp", ha, 0), ("grp", hb, 0), ("grp", ha, 1), ("grp", hb, 1),
                     ("emit_co", c)]
    schedule += [("grp", hb0, 0), ("grp", hb0, 1), ("emit_co", c0)]
    head_seq = []
    for step in schedule:
        if step[0] == "grp" and step[1] not in head_seq:
            head_seq.append(step[1])

    q_d = nc.dram_tensor("query", [BPC, HID], F32, kind="ExternalInput")
    k_d = nc.dram_tensor("key", [NPC, M, HEAD_DIM], F32, kind="ExternalInput")
    v_d = nc.dram_tensor("value", [NPC, M, HEAD_DIM], F32, kind="ExternalInput")
    wq_d = nc.dram_tensor("Wq", [HID, HID], F32, kind="ExternalInput")
    wo_d = nc.dram_tensor("Wo", [HID, HID], F32, kind="ExternalInput")
    kpe_d = nc.dram_tensor("key_pe", [HEAD_DIM, M], F32, kind="ExternalInput")
    span_d = nc.dram_tensor("span", [NHEADS, 1], F32, kind="ExternalInput")
    out_d = nc.dram_tensor("out", [BPC, HID], F32, kind="ExternalOutput")

    with TileContext(nc) as tc:
        with (
            tc.tile_pool(name="persist", bufs=1) as persist,
            # main-loop pools created BEFORE setup pools so the kv DMAs get
            # SBUF ranges disjoint from setup tiles (no WAR dep -> kv loads
            # start at t=0, overlapping the whole setup phase)
            tc.tile_pool(name="kv", bufs=4) as kv_pool,
            tc.tile_pool(name="sc", bufs=3) as sc_pool,
            tc.tile_pool(name="fin", bufs=1) as fin_pool,
            tc.tile_pool(name="ps_s", bufs=1, space="PSUM") as ps_s_pool,
            tc.tile_pool(name="ps_o", bufs=2, space="PSUM") as ps_o_pool,
            tc.tile_pool(name="ps_fin", bufs=1, space="PSUM") as ps_fin_pool,
        ):
            identity = persist.tile([128, 128], F32, tag="identity")
            make_identity(nc, identity[:])
            ones_row = persist.tile([1, 128], F32, tag="ones_row")
            nc.vector.memset(ones_row[:], 1.0)
            ones_col = persist.tile([128, 1], F32, tag="ones_col")
            nc.vector.memset(ones_col[:], 1.0)

            woT = [persist.tile([128, HID], F32, name=f"woT{j}", tag=f"woT{j}") for j in range(4)]
            q_sb = persist.tile([BPC, HID], F32, tag="q_sb")
            qts = persist.tile([HEAD_DIM, NHEADS, BPC], F32, tag="qts")
            qrep = persist.tile([128, BPC, HID], BF16, tag="qrep")
            pos_all = persist.tile([128, BPC, SUM_MO], F32, tag="pos_all")
            masks = persist.tile([128, SUM_MO], F32, tag="masks")
            ao_sb = persist.tile([1, BPC, HID], F32, tag="ao_sb")

            # wo_sb persists until the post-main-loop transposes; each tile
            # needs its OWN buffer (a rotating tag would make later wo DMAs
            # wait on post-loop consumers and block the scalar ring)
            wo_sb = [fin_pool.tile([128, HID], F32, name=f"wo_sb{i}", tag=f"wo_sb{i}") for i in range(4)]

            # ---------------- setup phase A: masks, Wq transposes, q --------
            with (
                tc.tile_pool(name="setupA", bufs=1) as sa,
                tc.tile_pool(name="psA", bufs=2, space="PSUM") as psA,
            ):
                # --- masks first: only needs span; gets ACT/DVE/GpSimd work
                # done before the transpose copies queue up
                span_row = sa.tile([1, NHEADS], F32, tag="span_row")
                nc.scalar.dma_start(out=span_row[:], in_=span_d[:].rearrange("h o -> o h"))
                ps_sp = psA.tile([128, NHEADS], F32, tag="pwt")
                nc.tensor.matmul(
                    ps_sp[:], ones_row[:], span_row[:], start=True, stop=True
                )
                # build the per-head affine mask grids from two tiny iotas
                # (8 fat gpsimd iotas had ~3us DRAINs each, delaying SWDGE
                # kv descriptor emission on the GpSimd engine)
                bias_const = float(-(M - 1) / RAMP + 1.0)  # -254.96875
                col_p = sa.tile([128, 1], F32, tag="col_p")
                nc.gpsimd.iota(
                    out=col_p[:], pattern=[[1, 1]], base=0,
                    channel_multiplier=1,
                    allow_small_or_imprecise_dtypes=True,
                )
                jgrid = sa.tile([128, max(mo_h)], F32, tag="jgrid")
                nc.gpsimd.iota(
                    out=jgrid[:], pattern=[[1, max(mo_h)]], base=0,
                    channel_multiplier=0,
                    allow_small_or_imprecise_dtypes=True,
                )
                span_b = sa.tile([128, NHEADS], F32, tag="span_b")
                pbias = sa.tile([128, NHEADS], F32, tag="pbias")
                for h in range(NHEADS):
                    # span_b[:,h] = span_h*256 - 254.97 + m0_h/32
                    nc.scalar.activation(
                        out=span_b[:, h : h + 1], in_=ps_sp[:, h : h + 1],
                        func=AF.Copy, scale=float(M / RAMP),
                        bias=float(bias_const + m0s[h] / RAMP),
                    )
                    # pbias[:,h] = mo_h*p/32 + span_b[:,h]
                    nc.scalar.activation(
                        out=pbias[:, h : h + 1], in_=col_p[:],
                        func=AF.Identity, scale=float(mo_h[h] / RAMP),
                        bias=span_b[:, h : h + 1],
                    )
                for h in range(NHEADS):
                    mk = masks[:, offs[h]:offs[h] + mo_h[h]]
                    nc.scalar.activation(
                        out=mk, in_=jgrid[:, 0:mo_h[h]], func=AF.Identity,
                        scale=float(1.0 / RAMP), bias=pbias[:, h : h + 1],
                    )
                    # clamp to [1e-30, 1]: keeps ln() finite (8*ln -> -553,
                    # exp -> ~1e-26 ~ 0) while matching clip(0,1) numerically
                    nc.vector.tensor_scalar(
                        out=mk, in0=mk,
                        scalar1=1e-30, scalar2=1.0,
                        op0=ALU.max, op1=ALU.min,
                    )
                # masks <- 8 * ln(mask): additive softmax bias
                nc.scalar.activation(
                    out=masks[:], in_=masks[:], func=AF.Ln,
                )
                nc.vector.tensor_scalar(
                    out=masks[:], in0=masks[:],
                    scalar1=float(math.sqrt(HEAD_DIM)), scalar2=0.0,
                    op0=ALU.mult, op1=ALU.add,
                )

                # --- Wq transposes + q projection ---
                wqT = [sa.tile([128, HID], F32, name=f"wqT{j}", tag=f"wqT{j}") for j in range(4)]
                wq_sb = [sa.tile([128, HID], F32, name=f"wq_sb{i}", tag=f"wq_sb{i}") for i in range(4)]
                for i in range(4):
                    nc.sync.dma_start(out=wq_sb[i][:], in_=wq_d[ts(i, 128), :])
                query_sb = sa.tile([BPC, HID], F32, tag="query_sb")
                nc.sync.dma_start(out=query_sb[:], in_=q_d[:])
                qTq = [sa.tile([128, BPC], F32, name=f"qTq{j}", tag=f"qTq{j}") for j in range(4)]
                for jo in range(4):
                    pqt = psA.tile([128, BPC], F32, tag="pwt")
                    nc.tensor.matmul(
                        pqt[:], query_sb[:, ts(jo, 128)], identity[0:BPC, 0:BPC],
                        start=True, stop=True,
                    )
                    nc.scalar.copy(qTq[jo][:], pqt[:])
                # q = query @ Wq.T, with each ps_q accumulation step emitted as
                # soon as its wqT[jo] chunk is transposed (jo-major order)
                ps_q = psA.tile([BPC, HID], F32, tag="ps_q", bufs=1)
                for jo in range(4):
                    for io in range(4):
                        pwt = psA.tile([128, 128], F32, tag="pwt")
                        nc.tensor.matmul(
                            pwt[:], wq_sb[io][:, ts(jo, 128)], identity[:],
                            start=True, stop=True,
                        )
                        nc.scalar.copy(wqT[jo][:, ts(io, 128)], pwt[:])
                    nc.tensor.matmul(
                        ps_q[:], qTq[jo][:], wqT[jo][:],
                        start=(jo == 0), stop=(jo == 3),
                    )
                nc.scalar.copy(q_sb[:], ps_q[:])
                # qts[d, h, b] = q[b, h*64+d]   (64 partitions)
                for h in range(NHEADS):
                    pqh = psA.tile([HEAD_DIM, BPC], F32, tag="pwt")
                    nc.tensor.matmul(
                        pqh[:], q_sb[:, ts(h, HEAD_DIM)], identity[0:BPC, 0:BPC],
                        start=True, stop=True,
                    )
                    nc.scalar.copy(qts[:, h, :], pqh[:])
                # qrep[p, b, :] = q[b, :] via PE broadcast: weights select
                # row b of q_sb into every output partition
                for b in range(BPC):
                    ps_rep = psA.tile([128, HID], F32, tag="pwt")
                    nc.tensor.matmul(
                        ps_rep[:],
                        identity[0:BPC, b : b + 1].broadcast_to((BPC, 128)),
                        q_sb[:],
                        start=True, stop=True,
                    )
                    nc.vector.tensor_copy(qrep[:, b, :], ps_rep[:])
                # Wo loads (scalar ring, after span; consumed post-main-loop)
                for i in range(4):
                    nc.scalar.dma_start(out=wo_sb[i][:], in_=wo_d[ts(i, 128), :])

            # ---------------- setup phase B: positional scores --------------
            with (
                tc.tile_pool(name="setupB", bufs=1) as sb,
                tc.tile_pool(name="psB", bufs=2, space="PSUM") as psB,
            ):
                # pos_nm[h*4+b, m] = sum_d q[(b,h),d] kpe[d,m]
                kpe_sb = sb.tile([HEAD_DIM, LALL], F32, tag="kpe_sb")
                nc.scalar.dma_start(out=kpe_sb[:], in_=kpe_d[:, m0_min:])
                pos_nm = sb.tile([NPC, LALL], F32, tag="pos_nm")
                qts_flat = qts[:].rearrange("d h b -> d (h b)")
                col = 0
                while col < LALL:
                    cw = min(512, LALL - col)
                    ps_pos = psB.tile([NPC, cw], F32, tag="ps_pos", bufs=2)
                    nc.tensor.matmul(
                        ps_pos[:], qts_flat, kpe_sb[:, col:col + cw],
                        start=True, stop=True,
                    )
                    nc.vector.tensor_copy(pos_nm[:, col:col + cw], ps_pos[:])
                    col += cw
                # re-grid per head into block layout: bounce via DRAM (DRAM
                # APs are pure strides, so the gather rearrange is legal) on
                # the scalar HWDGE ring, which carries no kv traffic
                with tc.tile_pool(name="dramp", bufs=1, space="DRAM") as dp:
                    pos_dram = dp.tile([NPC, LALL], F32, tag="pos_dram")
                    nc.scalar.dma_start(out=pos_dram[:], in_=pos_nm[:])
                    for h in head_seq:
                        nc.scalar.dma_start(
                            out=pos_all[:, :, offs[h]:offs[h] + mo_h[h]],
                            in_=pos_dram[ts(h, BPC), m0s[h] - m0_min:].rearrange(
                                "b (p mo) -> p b mo", mo=mo_h[h]
                            ),
                        )
                # the 8*ln(mask) fold into pos_all happens per head inside the
                # main loop, gated on that head's gather only, so the first
                # head's rows start ~8us earlier

            # ---------------- main loop over (h, b-group) -------------------
            # k_d/v_d rows are h-major (host permutation): row r = h*BPC + b.
            # One DMA fetches GB rows of a head; value is cast f32->bf16
            # in-flight by the SWDGE (gpsimd) DMA engine.
            co_done = {}
            posln_done = set()
            for step in schedule:
                if step[0] == "emit_co":
                    co = step[1]
                    ps_t2 = ps_fin_pool.tile([128, BPC], F32, name="ps_t2", tag="ps_t2", bufs=1)
                    for b in range(BPC):
                        nc.tensor.matmul(
                            ps_t2[:, b : b + 1],
                            ao_sb[0:1, b, ts(co, 128)],
                            identity[0:1, 0:1],
                            start=True, stop=True,
                        )
                    t_sb = fin_pool.tile([128, BPC], F32, name=f"t_sb{co}", tag=f"t_sb{co}")
                    nc.scalar.copy(t_sb[:], ps_t2[:])
                    co_done[co] = t_sb
                    io = len(co_done) - 1
                    for jo in range(4):
                        pwt2 = ps_fin_pool.tile([128, 128], F32, tag="ps_t2", bufs=1)
                        nc.tensor.matmul(
                            pwt2[:], wo_sb[io][:, ts(jo, 128)], identity[:],
                            start=True, stop=True,
                        )
                        nc.scalar.copy(woT[jo][:, ts(io, 128)], pwt2[:])
                    continue
                _, h, g = step
                if h not in posln_done:
                    posln_done.add(h)
                    sl = pos_all[:, :, offs[h]:offs[h] + mo_h[h]]
                    nc.vector.tensor_add(
                        sl, sl,
                        masks[:, offs[h]:offs[h] + mo_h[h]].rearrange(
                            "p (x j) -> p x j", x=1
                        ).broadcast_to((128, BPC, mo_h[h])),
                    )
                mo_n = mo_h[h]
                off = offs[h]
                r0 = h * BPC + g * GB
                kt2 = kv_pool.tile([128, GB, mo_n, HEAD_DIM], BF16, tag="kt", bufs=5)
                nc.gpsimd.dma_start(
                    out=kt2[:],
                    in_=k_d[r0:r0 + GB, m0s[h]:, :].rearrange(
                        "b (p mo) d -> p b mo d", p=128
                    ),
                )
                vtb2 = kv_pool.tile([128, GB, mo_n, HEAD_DIM], BF16, tag="vtb")
                nc.gpsimd.dma_start(
                    out=vtb2[:],
                    in_=v_d[r0:r0 + GB, m0s[h]:, :].rearrange(
                        "b (p mo) d -> p b mo d", p=128
                    ),
                )
                for bb in range(GB):
                    b = g * GB + bb
                    # content + positional scores:
                    # scores[p, mo] = pos[p, b, off+mo] + sum_d key[..] * q[(b,h), d]
                    prod = sc_pool.tile([128, mo_n, HEAD_DIM], BF16, tag="prod", bufs=1)
                    q_b = (
                        qrep[:, b, ts(h, HEAD_DIM)]
                        .rearrange("p (x d) -> p x d", x=1)
                        .broadcast_to((128, mo_n, HEAD_DIM))
                    )
                    nc.vector.tensor_mul(prod[:], kt2[:, bb], q_b)
                    scores = sc_pool.tile([128, mo_n], F32, tag="scores")
                    nc.vector.reduce_sum(scores[:], prod[:], axis=mybir.AxisListType.X)
                    nc.vector.tensor_add(scores[:], scores[:], pos_all[:, b, off:off + mo_n])
                    # w = exp((scores + pos + 8*ln(mask)) / sqrt(d)), Sigma_w fused
                    sums = sc_pool.tile([128, 1], F32, tag="sums")
                    w_t = sc_pool.tile([128, mo_n], BF16, tag="w_t")
                    nc.scalar.activation(
                        out=w_t[:], in_=scores[:], func=AF.Exp,
                        scale=float(1.0 / math.sqrt(HEAD_DIM)),
                        accum_out=sums[:],
                    )
                    # partition-reduce Sigma_w, then scal = 1/Sigma_w
                    ps_s = ps_s_pool.tile([1, 1], F32, tag="ps_s")
                    nc.tensor.matmul(
                        ps_s[:], ones_col[:], sums[:], start=True, stop=True
                    )
                    scal = sc_pool.tile([1, 1], F32, tag="scal")
                    nc.vector.reciprocal(scal[:], ps_s[:])
                    # out_row = sum_m w[m] * value[m, :]   (bf16 PE, PSUM accum)
                    ps_o = ps_o_pool.tile([1, HEAD_DIM], F32, tag="ps_o")
                    for mo in range(mo_n):
                        nc.tensor.matmul(
                            ps_o[:],
                            w_t[:, mo : mo + 1],
                            vtb2[:, bb, mo, :],
                            start=(mo == 0),
                            stop=(mo == mo_n - 1),
                        )
                    # ao[0, b, h*64:(h+1)*64] = ps_o * scal
                    nc.scalar.activation(
                        out=ao_sb[0:1, b, ts(h, HEAD_DIM)], in_=ps_o[:],
                        func=AF.Copy, scale=scal[:, 0:1],
                    )


            # ---------------- output projection -------------------------
            aoT = [co_done[co] for co in range(4)]
            ps_f = ps_fin_pool.tile([BPC, HID], F32, name="ps_f", tag="ps_fin")
            for co in range(4):
                nc.tensor.matmul(
                    ps_f[:], aoT[co][:], woT[co][:],
                    start=(co == 0), stop=(co == 3),
                )
            out_sb = fin_pool.tile([BPC, HID], F32, tag="out_sb")
            nc.scalar.copy(out_sb[:], ps_f[:])
            nc.sync.dma_start(out=out_d[:], in_=out_sb[:])

    nc.compile()
    return nc


def _get_nc(m0s):
    if m0s not in _CACHE:
        _CACHE[m0s] = build_nc(m0s)
    return _CACHE[m0s]


def _make_in_maps(query, key, value, Wq, Wo, key_pe, span):
    q2 = np.ascontiguousarray(np.asarray(query, np.float32).reshape(B, HID))
    key = np.asarray(key, np.float32)
    value = np.asarray(value, np.float32)
    Wq = np.ascontiguousarray(np.asarray(Wq, np.float32))
    Wo = np.ascontiguousarray(np.asarray(Wo, np.float32))
    key_pe = np.ascontiguousarray(np.asarray(key_pe, np.float32))
    span = np.ascontiguousarray(np.asarray(span, np.float32))
    # device key/value rows are h-major: row r = h*BPC + b
    perm = [b * NHEADS + h for h in range(NHEADS) for b in range(BPC)]
    in_maps = []
    for c in range(N_CORES):
        in_maps.append(
            {
                "query": np.ascontiguousarray(q2[c * BPC : (c + 1) * BPC]),
                "key": np.ascontiguousarray(key[c * NPC : (c + 1) * NPC][perm]),
                "value": np.ascontiguousarray(value[c * NPC : (c + 1) * NPC][perm]),
                "Wq": Wq,
                "Wo": Wo,
                "key_pe": key_pe,
                "span": span,
            }
        )
    return in_maps


def _install_ntff_hook():
    """Shim antenv.axon_hooks with a ctypes NTFF profile hook so
    run_bass_kernel_spmd(trace=True) works in this container."""
    import contextlib
    import ctypes
    import types

    try:
        import antenv.axon_hooks  # noqa: F401

        return
    except ImportError:
        pass
    so_path = "/opt/axon/libaxon_pjrt.so"
    import antenv

    mod = types.ModuleType("antenv.axon_hooks")
    holder = {"hook": None}

    if os.path.exists(so_path):
        lib = ctypes.CDLL(so_path)
        if hasattr(lib, "axon_start_nrt_profile"):
            lib.axon_start_nrt_profile.argtypes = [
                ctypes.POINTER(ctypes.c_int64),
                ctypes.c_size_t,
            ]
            lib.axon_start_nrt_profile.restype = ctypes.c_int64
            lib.axon_stop_nrt_profile.argtypes = [ctypes.c_char_p]
            lib.axon_stop_nrt_profile.restype = ctypes.c_int64

            @contextlib.contextmanager
            def _hook(output_dir, device_ids):
                import jax

                jax.devices()
                if device_ids:
                    ids = (ctypes.c_int64 * len(device_ids))(*device_ids)
                    rc = lib.axon_start_nrt_profile(ids, len(device_ids))
                else:
                    rc = lib.axon_start_nrt_profile(None, 0)
                if rc != 0:
                    raise RuntimeError(f"axon_start_nrt_profile rc={rc}")
                try:
                    yield
                finally:
                    n = lib.axon_stop_nrt_profile(str(output_dir).encode())
                    print(f"profile: {n} file(s) written to {output_dir}")

            holder["hook"] = _hook

    mod.get_axon_ntff_profile_hook = lambda: holder["hook"]
    mod.set_axon_ntff_profile_hook = lambda h: holder.__setitem__("hook", h)
    sys.modules["antenv.axon_hooks"] = mod
    antenv.axon_hooks = mod


def run(query, key, value, Wq, Wo, key_pe, span, trace=False):
    """Run on hardware; returns (output [B,1,HID], BassKernelResults)."""
    from concourse import bass_utils
    from concourse.bass_utils import run_bass_kernel_spmd

    if trace:
        _install_ntff_hook()
        bass_utils.upload_artifacts = lambda tmpdir: f"local:{tmpdir}"
    nc = _get_nc(_suffix_starts(span))
    in_maps = _make_in_maps(query, key, value, Wq, Wo, key_pe, span)
    res = run_bass_kernel_spmd(nc, in_maps, list(range(N_CORES)), trace=trace)
    out = np.concatenate(
        [np.asarray(res.results[c]["out"]) for c in range(N_CORES)], axis=0
    )
    return out.reshape(B, 1, HID).astype(np.float32), res


def kernel(query, key, value, Wq, Wo, key_pe, span):
    out, _ = run(query, key, value, Wq, Wo, key_pe, span, trace=False)
    return out


# revision 16
# speedup vs baseline: 1.1779x; 1.1779x over previous
"""Trainium2 Bass kernel for multi-head attention with adaptive span masking.

Computation (per the nn.Module):
    q = (query @ Wq.T) split into B*H rows of size d=64
    attn = softmax((key . q + q @ key_pe) / sqrt(d))
    attn = renormalize(attn * adaptive_span_mask)
    out = (attn . value) merged heads @ Wo.T

Sharding: batch-parallel across 8 cores. Core c gets batches [4c, 4c+4)
(all 8 heads) = rows [32c, 32c+32) of key/value; Wq/Wo/key_pe/span are
replicated. Each core produces its own [4, 512] output block; the host
concatenates. No collectives needed.

Sparsity: the adaptive-span mask is exactly zero for m <= 8159 - span*M,
so only the suffix [m0_h, M) of each head's key/value rows is ever used.
m0_h is computed on the host from the span input (any span values give a
correct kernel; new values just trigger a rebuild) and the kernel only
loads/processes that suffix. The mask enters as a precomputed additive
8*ln(mask) bias inside the exp (exact for mask>0; -inf -> weight 0), and
the 1e-8*sum(exp) regularizer of the reference is dropped (~1e-6 relative).

Positional scores are computed as one dense PE matmul q @ key_pe in
[row, m] layout, then re-gridded to each head's [128, mo_h] block layout
with per-head SBUF->SBUF gather DMAs (cheap; avoids hundreds of tiny
PE weight loads).
"""

import math
import os
import sys

import numpy as np

for _p in ("/opt/trn_rl_repo", "/root/.axon_site/_ro/trn_rl_repo"):
    if os.path.isdir(_p) and _p not in sys.path:
        sys.path.insert(0, _p)

import concourse.bass as bass
import concourse.bacc as bacc
import concourse.mybir as mybir
from concourse.bass import ts
from concourse.masks import make_identity
from concourse.tile import TileContext

F32 = mybir.dt.float32

# Problem constants (hardcoded per contest contract)
NHEADS = 8
HEAD_DIM = 64
HID = NHEADS * HEAD_DIM  # 512
B = 32
M = 8192
RAMP = 32.0

N_CORES = 8
BPC = B // N_CORES        # 4 batches per core
NPC = BPC * NHEADS        # 32 (b,h) rows per core

_CACHE = {}


def _suffix_starts(span):
    """Per-head start offset m0 such that mask[h, m] == 0 for all m < m0.

    mask = clip((m - (M-1) + span*M)/RAMP + 1, 0, 1) is zero iff
    m <= (M-1) - RAMP - span*M. Align down to 128 with a small safety
    margin for float rounding.
    """
    s = np.asarray(span, np.float64).ravel()
    m0 = np.floor((M - 1) - RAMP - s * M).astype(np.int64) - 2
    m0 = np.clip(m0, 0, M - 128)
    m0 = (m0 // 128) * 128
    return tuple(int(x) for x in m0)


def build_nc(m0s):
    nc = bacc.Bacc(None, target_bir_lowering=False)
    AF = mybir.ActivationFunctionType
    ALU = mybir.AluOpType
    BF16 = mybir.dt.bfloat16

    mo_h = [(M - m0) // 128 for m0 in m0s]   # per-head blocks of 128 rows
    offs = [0] * NHEADS                      # per-head column offset in pos/masks
    for h in range(1, NHEADS):
        offs[h] = offs[h - 1] + mo_h[h - 1]
    SUM_MO = offs[-1] + mo_h[-1]
    m0_min = min(m0s)
    LALL = M - m0_min

    # Host permutes key/value rows to h-major (row r = h*BPC + b), so GB
    # consecutive rows share a head and can be fetched with one batched DMA.
    # Group order: biggest groups first (prefetch covers setup), then
    # alternate big/small so small-group fixed compute hides under big-group
    # DMAs and the tail is not a run of tiny groups.
    GB = 2
    # co-pairs (heads 2j,2j+1 feed output column chunk j) ordered by total
    # size descending; each pair finishes before the next pair's tail so its
    # output transpose can run mid-loop instead of serializing the tail.
    co_order = sorted(range(4), key=lambda c: -max(mo_h[2 * c], mo_h[2 * c + 1]))
    # schedule = list of ("grp", h, g) / ("emit_co", c) steps. The biggest
    # pair's big head leads (prefetch depth); its small partner runs dead
    # last so the tail pipeline is short; other pairs complete mid-loop so
    # their output transposes overlap the stream.
    c0 = co_order[0]
    ha0, hb0 = 2 * c0, 2 * c0 + 1
    if mo_h[ha0] < mo_h[hb0]:
        ha0, hb0 = hb0, ha0
    schedule = [("grp", ha0, 0), ("grp", ha0, 1)]
    for c in co_order[1:]:
        ha, hb = 2 * c, 2 * c + 1
        if mo_h[ha] < mo_h[hb]:
            ha, hb = hb, ha
        schedule += [("grp", ha, 0), ("grp", hb, 0), ("grp", ha, 1), ("grp", hb, 1),
                     ("emit_co", c)]
    schedule += [("grp", hb0, 0), ("grp", hb0, 1), ("emit_co", c0)]
    head_seq = []
    for step in schedule:
        if step[0] == "grp" and step[1] not in head_seq:
            head_seq.append(step[1])

    q_d = nc.dram_tensor("query", [BPC, HID], F32, kind="ExternalInput")
    k_d = nc.dram_tensor("key", [NPC, M, HEAD_DIM], F32, kind="ExternalInput")
    v_d = nc.dram_tensor("value", [NPC, M, HEAD_DIM], F32, kind="ExternalInput")
    wq_d = nc.dram_tensor("Wq", [HID, HID], F32, kind="ExternalInput")
    wo_d = nc.dram_tensor("Wo", [HID, HID], F32, kind="ExternalInput")
    kpe_d = nc.dram_tensor("key_pe", [HEAD_DIM, M], F32, kind="ExternalInput")
    span_d = nc.dram_tensor("span", [NHEADS, 1], F32, kind="ExternalInput")
    out_d = nc.dram_tensor("out", [BPC, HID], F32, kind="ExternalOutput")

    with TileContext(nc) as tc:
        with (
            tc.tile_pool(name="persist", bufs=1) as persist,
            # main-loop pools created BEFORE setup pools so the kv DMAs get
            # SBUF ranges disjoint from setup tiles (no WAR dep -> kv loads
            # start at t=0, overlapping the whole setup phase)
            tc.tile_pool(name="kv", bufs=4) as kv_pool,
            tc.tile_pool(name="sc", bufs=3) as sc_pool,
            tc.tile_pool(name="fin", bufs=1) as fin_pool,
            tc.tile_pool(name="ps_s", bufs=2, space="PSUM") as ps_s_pool,
            tc.tile_pool(name="ps_o", bufs=2, space="PSUM") as ps_o_pool,
            tc.tile_pool(name="ps_fin", bufs=1, space="PSUM") as ps_fin_pool,
        ):
            identity = persist.tile([128, 128], F32, tag="identity")
            make_identity(nc, identity[:])
            ones_row = persist.tile([1, 128], F32, tag="ones_row")
            nc.vector.memset(ones_row[:], 1.0)
            ones_col = persist.tile([128, 1], F32, tag="ones_col")
            nc.vector.memset(ones_col[:], 1.0)

            woT = [persist.tile([128, HID], F32, name=f"woT{j}", tag=f"woT{j}") for j in range(4)]
            q_sb = persist.tile([BPC, HID], F32, tag="q_sb")
            qts = persist.tile([HEAD_DIM, NHEADS, BPC], F32, tag="qts")
            qrep = persist.tile([128, BPC, HID], BF16, tag="qrep")
            pos_all = persist.tile([128, BPC, SUM_MO], F32, tag="pos_all")
            masks = persist.tile([128, SUM_MO], F32, tag="masks")
            ao_sb = persist.tile([1, BPC, HID], F32, tag="ao_sb")

            # wo_sb persists until the post-main-loop transposes; each tile
            # needs its OWN buffer (a rotating tag would make later wo DMAs
            # wait on post-loop consumers and block the scalar ring)
            wo_sb = [fin_pool.tile([128, HID], F32, name=f"wo_sb{i}", tag=f"wo_sb{i}") for i in range(4)]

            # ---------------- setup phase A: masks, Wq transposes, q --------
            with (
                tc.tile_pool(name="setupA", bufs=1) as sa,
                tc.tile_pool(name="psA", bufs=2, space="PSUM") as psA,
            ):
                # --- masks first: only needs span; gets ACT/DVE/GpSimd work
                # done before the transpose copies queue up
                span_row = sa.tile([1, NHEADS], F32, tag="span_row")
                nc.scalar.dma_start(out=span_row[:], in_=span_d[:].rearrange("h o -> o h"))
                ps_sp = psA.tile([128, NHEADS], F32, tag="pwt")
                nc.tensor.matmul(
                    ps_sp[:], ones_row[:], span_row[:], start=True, stop=True
                )
                # build the per-head affine mask grids from two tiny iotas
                # (8 fat gpsimd iotas had ~3us DRAINs each, delaying SWDGE
                # kv descriptor emission on the GpSimd engine)
                bias_const = float(-(M - 1) / RAMP + 1.0)  # -254.96875
                col_p = sa.tile([128, 1], F32, tag="col_p")
                nc.gpsimd.iota(
                    out=col_p[:], pattern=[[1, 1]], base=0,
                    channel_multiplier=1,
                    allow_small_or_imprecise_dtypes=True,
                )
                jgrid = sa.tile([128, max(mo_h)], F32, tag="jgrid")
                nc.gpsimd.iota(
                    out=jgrid[:], pattern=[[1, max(mo_h)]], base=0,
                    channel_multiplier=0,
                    allow_small_or_imprecise_dtypes=True,
                )
                span_b = sa.tile([128, NHEADS], F32, tag="span_b")
                pbias = sa.tile([128, NHEADS], F32, tag="pbias")
                for h in range(NHEADS):
                    # span_b[:,h] = span_h*256 - 254.97 + m0_h/32
                    nc.scalar.activation(
                        out=span_b[:, h : h + 1], in_=ps_sp[:, h : h + 1],
                        func=AF.Copy, scale=float(M / RAMP),
                        bias=float(bias_const + m0s[h] / RAMP),
                    )
                    # pbias[:,h] = mo_h*p/32 + span_b[:,h]
                    nc.scalar.activation(
                        out=pbias[:, h : h + 1], in_=col_p[:],
                        func=AF.Identity, scale=float(mo_h[h] / RAMP),
                        bias=span_b[:, h : h + 1],
                    )
                for h in range(NHEADS):
                    mk = masks[:, offs[h]:offs[h] + mo_h[h]]
                    nc.scalar.activation(
                        out=mk, in_=jgrid[:, 0:mo_h[h]], func=AF.Identity,
                        scale=float(1.0 / RAMP), bias=pbias[:, h : h + 1],
                    )
                    # clamp to [1e-30, 1]: keeps ln() finite (8*ln -> -553,
                    # exp -> ~1e-26 ~ 0) while matching clip(0,1) numerically
                    nc.vector.tensor_scalar(
                        out=mk, in0=mk,
                        scalar1=1e-30, scalar2=1.0,
                        op0=ALU.max, op1=ALU.min,
                    )
                # masks <- 8 * ln(mask): additive softmax bias
                nc.scalar.activation(
                    out=masks[:], in_=masks[:], func=AF.Ln,
                )
                nc.vector.tensor_scalar(
                    out=masks[:], in0=masks[:],
                    scalar1=float(math.sqrt(HEAD_DIM)), scalar2=0.0,
                    op0=ALU.mult, op1=ALU.add,
                )

                # --- Wq transposes + q projection ---
                wqT = [sa.tile([128, HID], F32, name=f"wqT{j}", tag=f"wqT{j}") for j in range(4)]
                wq_sb = [sa.tile([128, HID], F32, name=f"wq_sb{i}", tag="wq_sb", bufs=2) for i in range(4)]
                for i in range(4):
                    nc.sync.dma_start(out=wq_sb[i][:], in_=wq_d[ts(i, 128), :])
                query_sb = sa.tile([BPC, HID], F32, tag="query_sb")
                nc.sync.dma_start(out=query_sb[:], in_=q_d[:])
                for io in range(4):
                    for jo in range(4):
                        pwt = psA.tile([128, 128], F32, tag="pwt")
                        nc.tensor.matmul(
                            pwt[:], wq_sb[io][:, ts(jo, 128)], identity[:],
                            start=True, stop=True,
                        )
                        nc.scalar.copy(wqT[jo][:, ts(io, 128)], pwt[:])
                qTq = [sa.tile([128, BPC], F32, name=f"qTq{j}", tag=f"qTq{j}") for j in range(4)]
                for jo in range(4):
                    pqt = psA.tile([128, BPC], F32, tag="pwt")
                    nc.tensor.matmul(
                        pqt[:], query_sb[:, ts(jo, 128)], identity[0:BPC, 0:BPC],
                        start=True, stop=True,
                    )
                    nc.scalar.copy(qTq[jo][:], pqt[:])
                # q = query @ Wq.T  ->  [4, 512]
                ps_q = psA.tile([BPC, HID], F32, tag="pwt")
                for jo in range(4):
                    nc.tensor.matmul(
                        ps_q[:], qTq[jo][:], wqT[jo][:],
                        start=(jo == 0), stop=(jo == 3),
                    )
                nc.scalar.copy(q_sb[:], ps_q[:])
                # qts[d, h, b] = q[b, h*64+d]   (64 partitions)
                for h in range(NHEADS):
                    pqh = psA.tile([HEAD_DIM, BPC], F32, tag="pwt")
                    nc.tensor.matmul(
                        pqh[:], q_sb[:, ts(h, HEAD_DIM)], identity[0:BPC, 0:BPC],
                        start=True, stop=True,
                    )
                    nc.scalar.copy(qts[:, h, :], pqh[:])
                # qrep[p, b, :] = q[b, :] via PE broadcast: weights select
                # row b of q_sb into every output partition
                for b in range(BPC):
                    ps_rep = psA.tile([128, HID], F32, tag="pwt")
                    nc.tensor.matmul(
                        ps_rep[:],
                        identity[0:BPC, b : b + 1].broadcast_to((BPC, 128)),
                        q_sb[:],
                        start=True, stop=True,
                    )
                    nc.vector.tensor_copy(qrep[:, b, :], ps_rep[:])
                # Wo loads (scalar ring, after span; consumed post-main-loop)
                for i in range(4):
                    nc.scalar.dma_start(out=wo_sb[i][:], in_=wo_d[ts(i, 128), :])

            # ---------------- setup phase B: positional scores --------------
            with (
                tc.tile_pool(name="setupB", bufs=1) as sb,
                tc.tile_pool(name="psB", bufs=2, space="PSUM") as psB,
            ):
                # pos_nm[h*4+b, m] = sum_d q[(b,h),d] kpe[d,m]
                kpe_sb = sb.tile([HEAD_DIM, LALL], F32, tag="kpe_sb")
                nc.scalar.dma_start(out=kpe_sb[:], in_=kpe_d[:, m0_min:])
                pos_nm = sb.tile([NPC, LALL], F32, tag="pos_nm")
                qts_flat = qts[:].rearrange("d h b -> d (h b)")
                col = 0
                while col < LALL:
                    cw = min(512, LALL - col)
                    ps_pos = psB.tile([NPC, cw], F32, tag="ps_pos", bufs=2)
                    nc.tensor.matmul(
                        ps_pos[:], qts_flat, kpe_sb[:, col:col + cw],
                        start=True, stop=True,
                    )
                    nc.vector.tensor_copy(pos_nm[:, col:col + cw], ps_pos[:])
                    col += cw
                # re-grid per head into block layout: bounce via DRAM (DRAM
                # APs are pure strides, so the gather rearrange is legal) on
                # the scalar HWDGE ring, which carries no kv traffic
                with tc.tile_pool(name="dramp", bufs=1, space="DRAM") as dp:
                    pos_dram = dp.tile([NPC, LALL], F32, tag="pos_dram")
                    nc.scalar.dma_start(out=pos_dram[:], in_=pos_nm[:])
                    for h in head_seq:
                        nc.scalar.dma_start(
                            out=pos_all[:, :, offs[h]:offs[h] + mo_h[h]],
                            in_=pos_dram[ts(h, BPC), m0s[h] - m0_min:].rearrange(
                                "b (p mo) -> p b mo", mo=mo_h[h]
                            ),
                        )
                # 8*ln(mask) folds into pos_all per head inside the main loop,
                # gated on that head's own gather, so the first head's rows
                # start ~8us earlier

            # ---------------- main loop over (h, b-group) -------------------
            # k_d/v_d rows are h-major (host permutation): row r = h*BPC + b.
            # One DMA fetches GB rows of a head; value is cast f32->bf16
            # in-flight by the SWDGE (gpsimd) DMA engine.
            co_done = {}
            posln_done = set()
            for step in schedule:
                if step[0] == "emit_co":
                    co = step[1]
                    ps_t2 = ps_fin_pool.tile([128, BPC], F32, name="ps_t2", tag="ps_t2", bufs=1)
                    for b in range(BPC):
                        nc.tensor.matmul(
                            ps_t2[:, b : b + 1],
                            ao_sb[0:1, b, ts(co, 128)],
                            identity[0:1, 0:1],
                            start=True, stop=True,
                        )
                    t_sb = fin_pool.tile([128, BPC], F32, name=f"t_sb{co}", tag=f"t_sb{co}")
                    nc.scalar.copy(t_sb[:], ps_t2[:])
                    co_done[co] = t_sb
                    io = len(co_done) - 1
                    for jo in range(4):
                        pwt2 = ps_fin_pool.tile([128, 128], F32, tag="ps_t2", bufs=1)
                        nc.tensor.matmul(
                            pwt2[:], wo_sb[io][:, ts(jo, 128)], identity[:],
                            start=True, stop=True,
                        )
                        nc.scalar.copy(woT[jo][:, ts(io, 128)], pwt2[:])
                    continue
                _, h, g = step
                if h not in posln_done:
                    posln_done.add(h)
                    sl = pos_all[:, :, offs[h]:offs[h] + mo_h[h]]
                    nc.vector.tensor_add(
                        sl, sl,
                        masks[:, offs[h]:offs[h] + mo_h[h]].rearrange(
                            "p (x j) -> p x j", x=1
                        ).broadcast_to((128, BPC, mo_h[h])),
                    )
                mo_n = mo_h[h]
                off = offs[h]
                r0 = h * BPC + g * GB
                kt2 = kv_pool.tile([128, GB, mo_n, HEAD_DIM], BF16, tag="kt", bufs=5)
                nc.gpsimd.dma_start(
                    out=kt2[:],
                    in_=k_d[r0:r0 + GB, m0s[h]:, :].rearrange(
                        "b (p mo) d -> p b mo d", p=128
                    ),
                )
                vtb2 = kv_pool.tile([128, GB, mo_n, HEAD_DIM], BF16, tag="vtb")
                nc.gpsimd.dma_start(
                    out=vtb2[:],
                    in_=v_d[r0:r0 + GB, m0s[h]:, :].rearrange(
                        "b (p mo) d -> p b mo d", p=128
                    ),
                )
                for bb in range(GB):
                    b = g * GB + bb
                    # content + positional scores:
                    # scores[p, mo] = pos[p, b, off+mo] + sum_d key[..] * q[(b,h), d]
                    prod = sc_pool.tile([128, mo_n, HEAD_DIM], BF16, tag="prod", bufs=1)
                    q_b = (
                        qrep[:, b, ts(h, HEAD_DIM)]
                        .rearrange("p (x d) -> p x d", x=1)
                        .broadcast_to((128, mo_n, HEAD_DIM))
                    )
                    nc.vector.tensor_mul(prod[:], kt2[:, bb], q_b)
                    scores = sc_pool.tile([128, mo_n], F32, tag="scores")
                    nc.vector.reduce_sum(scores[:], prod[:], axis=mybir.AxisListType.X)
                    nc.vector.tensor_add(scores[:], scores[:], pos_all[:, b, off:off + mo_n])
                    # w = exp((scores + pos + 8*ln(mask)) / sqrt(d)), Sigma_w fused
                    sums = sc_pool.tile([128, 1], F32, tag="sums")
                    w_t = sc_pool.tile([128, mo_n], BF16, tag="w_t")
                    nc.scalar.activation(
                        out=w_t[:], in_=scores[:], func=AF.Exp,
                        scale=float(1.0 / math.sqrt(HEAD_DIM)),
                        accum_out=sums[:],
                    )
                    # partition-reduce Sigma_w, then scal = 1/Sigma_w
                    ps_s = ps_s_pool.tile([1, 1], F32, tag="ps_s")
                    nc.tensor.matmul(
                        ps_s[:], ones_col[:], sums[:], start=True, stop=True
                    )
                    scal = sc_pool.tile([1, 1], F32, tag="scal")
                    nc.vector.reciprocal(scal[:], ps_s[:])
                    # out_row = sum_m w[m] * value[m, :]   (bf16 PE, PSUM accum)
                    ps_o = ps_o_pool.tile([1, HEAD_DIM], F32, tag="ps_o")
                    for mo in range(mo_n):
                        nc.tensor.matmul(
                            ps_o[:],
                            w_t[:, mo : mo + 1],
                            vtb2[:, bb, mo, :],
                            start=(mo == 0),
                            stop=(mo == mo_n - 1),
                        )
                    # ao[0, b, h*64:(h+1)*64] = ps_o * scal
                    nc.scalar.activation(
                        out=ao_sb[0:1, b, ts(h, HEAD_DIM)], in_=ps_o[:],
                        func=AF.Copy, scale=scal[:, 0:1],
                    )


            # ---------------- output projection -------------------------
            aoT = [co_done[co] for co in range(4)]
            ps_f = ps_fin_pool.tile([BPC, HID], F32, name="ps_f", tag="ps_fin")
            for co in range(4):
                nc.tensor.matmul(
                    ps_f[:], aoT[co][:], woT[co][:],
                    start=(co == 0), stop=(co == 3),
                )
            out_sb = fin_pool.tile([BPC, HID], F32, tag="out_sb")
            nc.scalar.copy(out_sb[:], ps_f[:])
            nc.sync.dma_start(out=out_d[:], in_=out_sb[:])

    nc.compile()
    return nc


def _get_nc(m0s):
    if m0s not in _CACHE:
        _CACHE[m0s] = build_nc(m0s)
    return _CACHE[m0s]


def _make_in_maps(query, key, value, Wq, Wo, key_pe, span):
    q2 = np.ascontiguousarray(np.asarray(query, np.float32).reshape(B, HID))
    key = np.asarray(key, np.float32)
    value = np.asarray(value, np.float32)
    Wq = np.ascontiguousarray(np.asarray(Wq, np.float32))
    Wo = np.ascontiguousarray(np.asarray(Wo, np.float32))
    key_pe = np.ascontiguousarray(np.asarray(key_pe, np.float32))
    span = np.ascontiguousarray(np.asarray(span, np.float32))
    # device key/value rows are h-major: row r = h*BPC + b
    perm = [b * NHEADS + h for h in range(NHEADS) for b in range(BPC)]
    in_maps = []
    for c in range(N_CORES):
        in_maps.append(
            {
                "query": np.ascontiguousarray(q2[c * BPC : (c + 1) * BPC]),
                "key": np.ascontiguousarray(key[c * NPC : (c + 1) * NPC][perm]),
                "value": np.ascontiguousarray(value[c * NPC : (c + 1) * NPC][perm]),
                "Wq": Wq,
                "Wo": Wo,
                "key_pe": key_pe,
                "span": span,
            }
        )
    return in_maps


def _install_ntff_hook():
    """Shim antenv.axon_hooks with a ctypes NTFF profile hook so
    run_bass_kernel_spmd(trace=True) works in this container."""
    import contextlib
    import ctypes
    import types

    try:
        import antenv.axon_hooks  # noqa: F401

        return
    except ImportError:
        pass
    so_path = "/opt/axon/libaxon_pjrt.so"
    import antenv

    mod = types.ModuleType("antenv.axon_hooks")
    holder = {"hook": None}

    if os.path.exists(so_path):
        lib = ctypes.CDLL(so_path)
        if hasattr(lib, "axon_start_nrt_profile"):
            lib.axon_start_nrt_profile.argtypes = [
                ctypes.POINTER(ctypes.c_int64),
                ctypes.c_size_t,
            ]
            lib.axon_start_nrt_profile.restype = ctypes.c_int64
            lib.axon_stop_nrt_profile.argtypes = [ctypes.c_char_p]
            lib.axon_stop_nrt_profile.restype = ctypes.c_int64

            @contextlib.contextmanager
            def _hook(output_dir, device_ids):
                import jax

                jax.devices()
                if device_ids:
                    ids = (ctypes.c_int64 * len(device_ids))(*device_ids)
                    rc = lib.axon_start_nrt_profile(ids, len(device_ids))
                else:
                    rc = lib.axon_start_nrt_profile(None, 0)
                if rc != 0:
                    raise RuntimeError(f"axon_start_nrt_profile rc={rc}")
                try:
                    yield
                finally:
                    n = lib.axon_stop_nrt_profile(str(output_dir).encode())
                    print(f"profile: {n} file(s) written to {output_dir}")

            holder["hook"] = _hook

    mod.get_axon_ntff_profile_hook = lambda: holder["hook"]
    mod.set_axon_ntff_profile_hook = lambda h: holder.__setitem__("hook", h)
    sys.modules["antenv.axon_hooks"] = mod
    antenv.axon_hooks = mod


def run(query, key, value, Wq, Wo, key_pe, span, trace=False):
    """Run on hardware; returns (output [B,1,HID], BassKernelResults)."""
    from concourse import bass_utils
    from concourse.bass_utils import run_bass_kernel_spmd

    if trace:
        _install_ntff_hook()
        bass_utils.upload_artifacts = lambda tmpdir: f"local:{tmpdir}"
    nc = _get_nc(_suffix_starts(span))
    in_maps = _make_in_maps(query, key, value, Wq, Wo, key_pe, span)
    res = run_bass_kernel_spmd(nc, in_maps, list(range(N_CORES)), trace=trace)
    out = np.concatenate(
        [np.asarray(res.results[c]["out"]) for c in range(N_CORES)], axis=0
    )
    return out.reshape(B, 1, HID).astype(np.float32), res


def kernel(query, key, value, Wq, Wo, key_pe, span):
    out, _ = run(query, key, value, Wq, Wo, key_pe, span, trace=False)
    return out
